# revision 4
# baseline (speedup 1.0000x reference)
"""Trainium2 Bass kernel for nn_AdGraphAttn: multi-head edge attention GNN
with global adaptive attention pooling, SPMD across 8 NeuronCores.

Strategy: nodes are sharded across the 8 cores (degree-balanced, 49 tiles of
128 node slots per core); edges are partitioned by destination node and packed
into 8 subtiles of 128 edge slots per node tile.  Instead of gathering k/v
rows (DMA gather is descriptor-bound on this part), the host stages the x row
of every edge's source next to the owning destination tile, and the kernel
projects k/v per edge slot on the TensorEngine.  Per-destination softmax is
computed without max subtraction (scores are O(6)); segment sums use one-hot
indicator matmuls accumulated in PSUM.  The global-pool softmax and the pooled
output y are reduced on the host from per-core partial sums (exp is shift
invariant, so no cross-core max/sum exchange is needed on device).
"""

import sys
import heapq

sys.path.insert(0, "/opt/trn_rl_repo")

import numpy as np
import ml_dtypes

import concourse.bass as bass
import concourse.mybir as mybir
import concourse.tile as tile
from concourse.tile import TileContext, ScopedClock
from concourse.bass_utils import run_bass_kernel_spmd
from concourse.masks import make_identity

N_CORES = 8
N, E, L, D, H, NCL = 50000, 400000, 1024, 256, 4, 2
C = D // H
NT = 49            # node tiles per core
P = 128            # node slots per tile / partition count
SLOTS = NT * P     # 6272 node slots per core

bf16 = mybir.dt.bfloat16
f32 = mybir.dt.float32
AF = mybir.ActivationFunctionType
ALU = mybir.AluOpType


# --------------------------------------------------------------------------
# Tile-framework compatibility patches for this walrus build: instructions may
# carry at most ONE semaphore wait; split extra waits onto preceding no-ops.
# --------------------------------------------------------------------------
def _drain_and_barrier(self, tick_clock, wait_clock):
    drain_inst = self.nc.sync.drain()
    wait_clock.add_sem_waits(drain_inst.ins, ScopedClock({None: tick_clock.global_clock}))
    si = drain_inst.ins.sync_info
    waits = list(si.on_wait)
    if len(waits) > 1:
        si.on_wait.clear()
        si.on_wait.append(waits[0])
        for w in waits[1:]:
            d2 = self.nc.sync.drain()
            d2.ins.sync_info = mybir.SyncInfo(on_wait=[w], on_update=[])
    self.nc.all_engine_barrier()
    popped = self.nc._tile_sem_poison_stack.pop()
    assert popped is self._sem_poison
    self.nc.clear_and_free_semaphores(list(self.sems.allocated().values()))
    self.nc.all_engine_barrier()


TileContext._drain_and_barrier = _drain_and_barrier


def split_multi_waits(nc):
    for f in nc.m.functions:
        for bb in f.blocks:
            new_insts = []
            for inst in bb.instructions:
                si = getattr(inst, "sync_info", None)
                if si is not None and si.on_wait and len(si.on_wait) > 1:
                    waits = list(si.on_wait)
                    for j, w in enumerate(waits[:-1]):
                        new_insts.append(mybir.InstNoOp(
                            name=f"{inst.name}-sw{j}", engine=inst.engine,
                            bass_nofuse=True,
                            sync_info=mybir.SyncInfo(on_wait=[w], on_update=[])))
                    si.on_wait.clear()
                    si.on_wait.append(waits[-1])
                new_insts.append(inst)
            bb.instructions[:] = new_insts


# --------------------------------------------------------------------------
# Host-side graph partitioning
# --------------------------------------------------------------------------
def build_assignment(edge_index):
    """Degree-balanced node->tile->core assignment plus per-tile edge slots.

    Returns dict with:
      G            edge subtiles per node tile
      slot_node    [N_CORES, SLOTS] original node id per slot (-1 = dummy)
      src_ids      [N_CORES, NT, 128 + G*128] x-row id per staged column
                   (col 0:128 own nodes, then edge sources; -1 = zero row)
      dst_rel      [N_CORES, 128, NT*G] float32 slot of each edge's dst
                   within its tile (-1 = dummy edge slot)
      n_dummy      [N_CORES] dummy node slots per core (for Z correction)
    """
    src, dst = np.asarray(edge_index[0]), np.asarray(edge_index[1])
    deg = np.bincount(dst, minlength=N)
    n_tiles = N_CORES * NT

    order = np.argsort(-deg, kind="stable")
    tile_of = np.empty(N, np.int32)
    tile_deg = np.zeros(n_tiles, np.int64)
    tile_cnt = np.zeros(n_tiles, np.int32)
    heap = [(0, t) for t in range(n_tiles)]
    heapq.heapify(heap)
    for n in order:
        while True:
            _, t = heapq.heappop(heap)
            if tile_cnt[t] < P:
                break
        tile_of[n] = t
        tile_cnt[t] += 1
        tile_deg[t] += deg[n]
        if tile_cnt[t] < P:
            heapq.heappush(heap, (int(tile_deg[t]), t))

    G = max(8, int(np.ceil(tile_deg.max() / P)))

    # tiles -> cores, balancing edge counts
    t_order = np.argsort(-tile_deg)
    core_of_tile = np.empty(n_tiles, np.int32)
    core_load = np.zeros(N_CORES, np.int64)
    core_ntiles = np.zeros(N_CORES, np.int32)
    heap2 = [(0, c) for c in range(N_CORES)]
    heapq.heapify(heap2)
    for t in t_order:
        while True:
            _, c = heapq.heappop(heap2)
            if core_ntiles[c] < NT:
                break
        core_of_tile[t] = c
        core_ntiles[c] += 1
        core_load[c] += tile_deg[t]
        if core_ntiles[c] < NT:
            heapq.heappush(heap2, (int(core_load[c]), c))

    # local tile index per core
    local_tile = np.empty(n_tiles, np.int32)
    nxt = np.zeros(N_CORES, np.int32)
    for t in range(n_tiles):
        c = core_of_tile[t]
        local_tile[t] = nxt[c]
        nxt[c] += 1

    # node -> (core, tile, slot)
    slot_node = np.full((N_CORES, SLOTS), -1, np.int64)
    slot_in_tile = np.zeros(n_tiles, np.int32)
    node_core = np.empty(N, np.int32)
    node_slot = np.empty(N, np.int32)  # slot within tile (0..127)
    node_ltile = np.empty(N, np.int32)
    for n in range(N):
        t = tile_of[n]
        c = core_of_tile[t]
        lt = local_tile[t]
        s = slot_in_tile[t]
        slot_in_tile[t] += 1
        node_core[n] = c
        node_ltile[n] = lt
        node_slot[n] = s
        slot_node[c, lt * P + s] = n

    # group edges by (core, local tile)
    e_core = node_core[dst]
    e_tile = node_ltile[dst]
    key = (e_core.astype(np.int64) * NT + e_tile)
    eorder = np.argsort(key, kind="stable")
    key_s = key[eorder]
    src_s = src[eorder]
    dslot_s = node_slot[dst][eorder]
    starts = np.searchsorted(key_s, np.arange(N_CORES * NT))
    ends = np.searchsorted(key_s, np.arange(N_CORES * NT), side="right")

    EC = G * P                       # edge slots per node tile
    src_ids = np.full((N_CORES, NT, P + EC), -1, np.int64)
    dst_rel = np.full((N_CORES, NT * G, P), -1.0, np.float32)
    for c in range(N_CORES):
        for lt in range(NT):
            k = c * NT + lt
            a, b = starts[k], ends[k]
            cnt = b - a
            assert cnt <= EC, f"tile {k} has {cnt} edges > {EC}"
            src_ids[c, lt, 0:P] = slot_node[c, lt * P:(lt + 1) * P]
            src_ids[c, lt, P:P + cnt] = src_s[a:b]
            # dummy edge slots keep src -1 (zero row)
            dr = dst_rel[c, lt * G:(lt + 1) * G].reshape(EC)
            dr[:cnt] = dslot_s[a:b].astype(np.float32)
    # dst_rel layout: [core, subtile, slot] -> transpose to [core, 128, NT*G]
    dst_rel = np.ascontiguousarray(dst_rel.transpose(0, 2, 1))
    n_dummy = (slot_node == -1).sum(axis=1)
    return dict(G=G, slot_node=slot_node, src_ids=src_ids, dst_rel=dst_rel,
                n_dummy=n_dummy)


def stage_core_inputs(asg, x_bf16_ext, Wqs, Wkv, GP, dst_rel_c, dstt_c, src_ids_c, G):
    """Build one core's input map.  x_bf16_ext has an extra zero row at index N."""
    cols = src_ids_c.reshape(-1)          # [NT*(P+G*128)]
    idx = np.where(cols < 0, N, cols)
    xr = x_bf16_ext[idx]                  # [NT*(128+G*128), 1024]
    ncols = xr.shape[0]
    # -> [8, 128, ncols]: xsl[l, p, col] = xr[col, l*128+p]
    xsl = np.ascontiguousarray(xr.reshape(ncols, 8, P).transpose(1, 2, 0))
    return {
        "xsl": xsl,
        "wqs": Wqs, "wkv": Wkv, "gp": GP,
        "dstrel": dst_rel_c, "dstt": dstt_c,
    }


# --------------------------------------------------------------------------
# Device kernel builder
# --------------------------------------------------------------------------
def build_nc(G):
    EC = G * P
    CPT = P + EC                         # staged x columns per node tile
    nc = bass.Bass("TRN2", target_bir_lowering=False, debug=False)

    xsl_in = nc.dram_tensor("xsl", [8, P, NT * CPT], bf16, kind="ExternalInput")
    wqs_in = nc.dram_tensor("wqs", [L, 2 * D], bf16, kind="ExternalInput")
    wkv_in = nc.dram_tensor("wkv", [L, 2 * D], bf16, kind="ExternalInput")
    gp_in = nc.dram_tensor("gp", [D, D + NCL], f32, kind="ExternalInput")
    dstrel_in = nc.dram_tensor("dstrel", [P, NT * G], f32, kind="ExternalInput")
    dstt_in = nc.dram_tensor("dstt", [P, NT * EC], bf16, kind="ExternalInput")

    h_out = nc.dram_tensor("h", [SLOTS, D], f32, kind="ExternalOutput")
    es_out = nc.dram_tensor("es", [SLOTS, NCL], f32, kind="ExternalOutput")
    yu_out = nc.dram_tensor("yu", [2, D + 1], f32, kind="ExternalOutput")

    with tile.TileContext(nc) as tc:
        with (
            tc.tile_pool(name="const", bufs=1) as constp,
            tc.tile_pool(name="xtile", bufs=2) as xpool,
            tc.tile_pool(name="work", bufs=3) as work,
            tc.tile_pool(name="ps_big", bufs=3, space="PSUM") as ps_big,
            tc.tile_pool(name="ps_small", bufs=2, space="PSUM") as ps_small,
            tc.tile_pool(name="ps_acc", bufs=2, space="PSUM") as ps_acc,
            tc.tile_pool(name="ps_yu", bufs=1, space="PSUM") as ps_yu,
        ):
            ident_f = constp.tile([P, P], f32)
            make_identity(nc, ident_f[:])
            iota_b = constp.tile([P, P], bf16)
            nc.gpsimd.iota(iota_b[:], pattern=[[1, P]], base=0,
                           channel_multiplier=0,
                           allow_small_or_imprecise_dtypes=True)
            iota_p = constp.tile([P, 1], f32)
            nc.gpsimd.iota(iota_p[:], pattern=[[0, 1]], base=0,
                           channel_multiplier=1,
                           allow_small_or_imprecise_dtypes=True)
            ones_sb = constp.tile([P, 1], f32)
            nc.gpsimd.memset(ones_sb[:], 1.0)

            wqs_res = constp.tile([P, 8, 2 * D], bf16)
            wkv_res = constp.tile([P, 8, 2 * D], bf16)
            for l in range(8):
                nc.sync.dma_start(wqs_res[:, l, :], wqs_in.ap()[l * P:(l + 1) * P, :])
                nc.sync.dma_start(wkv_res[:, l, :], wkv_in.ap()[l * P:(l + 1) * P, :])
            gp_res = constp.tile([P, 2, D + NCL], f32)
            for c2 in range(2):
                nc.sync.dma_start(gp_res[:, c2, :], gp_in.ap()[c2 * P:(c2 + 1) * P, :])
            dstrel_res = constp.tile([P, NT * G], f32)
            nc.sync.dma_start(dstrel_res[:], dstrel_in.ap()[:])

            yu_ps = ps_yu.tile([2, D + 1], f32, space="PSUM")

            for nt in range(NT):
                # ---- staged x slab + transposed-dst slab for this node tile
                xt = xpool.tile([P, 8, CPT], bf16, tag="xt")
                for l in range(8):
                    nc.sync.dma_start(
                        xt[:, l, :], xsl_in.ap()[l, :, nt * CPT:(nt + 1) * CPT])
                dstt = xpool.tile([P, EC], bf16, tag="dstt")
                nc.sync.dma_start(dstt[:], dstt_in.ap()[:, nt * EC:(nt + 1) * EC])

                # ---- own projections: q | skip
                qs_ps = ps_big.tile([P, 2 * D], f32, space="PSUM", tag="big")
                for l in range(8):
                    nc.tensor.matmul(qs_ps[:], lhsT=xt[:, l, 0:P],
                                     rhs=wqs_res[:, l, :],
                                     start=(l == 0), stop=(l == 7))
                qskip = work.tile([P, 2 * D], bf16, tag="qskip")
                nc.scalar.copy(qskip[:], qs_ps[:])

                # ---- fused edge subtiles
                acc = ps_acc.tile([P, D + H], f32, space="PSUM", tag="acc")
                for g in range(G):
                    kv_ps = ps_big.tile([P, 2 * D], f32, space="PSUM", tag="big")
                    for l in range(8):
                        nc.tensor.matmul(
                            kv_ps[:],
                            lhsT=xt[:, l, P + g * P:P + (g + 1) * P],
                            rhs=wkv_res[:, l, :],
                            start=(l == 0), stop=(l == 7))
                    ind = work.tile([P, P], bf16, tag="ind")
                    nc.vector.tensor_scalar(
                        out=ind[:], in0=iota_b[:],
                        scalar1=dstrel_res[:, nt * G + g:nt * G + g + 1],
                        scalar2=None, op0=ALU.is_equal)
                    indT = work.tile([P, P], bf16, tag="indT")
                    nc.vector.tensor_scalar(
                        out=indT[:], in0=dstt[:, g * P:(g + 1) * P],
                        scalar1=iota_p[:], scalar2=None, op0=ALU.is_equal)
                    qd_ps = ps_small.tile([P, D], f32, space="PSUM", tag="small")
                    nc.tensor.matmul(qd_ps[:], lhsT=indT[:], rhs=qskip[:, 0:D],
                                     start=True, stop=True)
                    qd = work.tile([P, D], bf16, tag="qd")
                    nc.scalar.copy(qd[:], qd_ps[:])
                    prod = work.tile([P, D], bf16, tag="prod")
                    nc.vector.tensor_tensor(out=prod[:], in0=qd[:],
                                            in1=kv_ps[:, 0:D], op=ALU.mult)
                    s_g = work.tile([P, H], f32, tag="sg")
                    nc.vector.tensor_reduce(
                        out=s_g[:],
                        in_=prod[:].rearrange("p (h c) -> p h c", h=H),
                        axis=mybir.AxisListType.X, op=ALU.add)
                    ex_g = work.tile([P, H], f32, tag="exg")
                    nc.scalar.activation(ex_g[:], s_g[:], AF.Exp,
                                         scale=1.0 / np.sqrt(C))
                    M = work.tile([P, D + H], bf16, tag="M")
                    for h in range(H):
                        nc.vector.tensor_scalar(
                            out=M[:, h * C:(h + 1) * C],
                            in0=kv_ps[:, D + h * C:D + (h + 1) * C],
                            scalar1=ex_g[:, h:h + 1], scalar2=None,
                            op0=ALU.mult)
                    nc.vector.tensor_copy(M[:, D:D + H], ex_g[:])
                    nc.tensor.matmul(acc[:], lhsT=ind[:], rhs=M[:],
                                     start=(g == 0), stop=(g == G - 1))

                # ---- h = gelu(num/den + skip)
                den = work.tile([P, H], f32, tag="den")
                nc.vector.tensor_scalar(out=den[:], in0=acc[:, D:D + H],
                                        scalar1=1e-30, scalar2=None, op0=ALU.max)
                den_r = work.tile([P, H], f32, tag="denr")
                nc.vector.reciprocal(den_r[:], den[:])
                hpre = work.tile([P, D], f32, tag="hpre")
                for h in range(H):
                    nc.vector.tensor_scalar(
                        out=hpre[:, h * C:(h + 1) * C],
                        in0=acc[:, h * C:(h + 1) * C],
                        scalar1=den_r[:, h:h + 1], scalar2=None, op0=ALU.mult)
                nc.vector.tensor_tensor(out=hpre[:], in0=hpre[:],
                                        in1=qskip[:, D:2 * D], op=ALU.add)
                h_t = work.tile([P, D], f32, tag="ht")
                nc.scalar.activation(h_t[:], hpre[:], AF.Gelu)
                nc.sync.dma_start(h_out.ap()[nt * P:(nt + 1) * P, :], h_t[:])

                # ---- global pool partials
                hT = work.tile([P, 2, P], f32, tag="hT")
                for c2 in range(2):
                    hT_ps = ps_small.tile([P, P], f32, space="PSUM", tag="small")
                    nc.tensor.transpose(out=hT_ps[:], in_=h_t[:, c2 * P:(c2 + 1) * P],
                                        identity=ident_f[:])
                    nc.vector.tensor_copy(hT[:, c2, :], hT_ps[:])
                sp_ps = ps_big.tile([P, D + NCL], f32, space="PSUM", tag="big")
                for c2 in range(2):
                    nc.tensor.matmul(sp_ps[:], lhsT=hT[:, c2, :], rhs=gp_res[:, c2, :],
                                     start=(c2 == 0), stop=(c2 == 1))
                es = work.tile([P, NCL], f32, tag="es")
                nc.scalar.activation(es[:], sp_ps[:, D:D + NCL], AF.Exp)
                nc.sync.dma_start(es_out.ap()[nt * P:(nt + 1) * P, :], es[:])
                p_sb = work.tile([P, D + 1], f32, tag="p")
                nc.scalar.copy(p_sb[:, 0:D], sp_ps[:, 0:D])
                nc.vector.tensor_copy(p_sb[:, D:D + 1], ones_sb[:])
                nc.tensor.matmul(yu_ps[:], lhsT=es[:], rhs=p_sb[:],
                                 start=(nt == 0), stop=(nt == NT - 1))

            yu_sb = constp.tile([2, D + 1], f32)
            nc.vector.tensor_copy(yu_sb[:], yu_ps[:])
            nc.sync.dma_start(yu_out.ap()[:], yu_sb[:])

    split_multi_waits(nc)
    return nc


# --------------------------------------------------------------------------
# Full pipeline
# --------------------------------------------------------------------------
def prepare(inputs):
    x = np.asarray(inputs["x"], np.float32)
    asg = build_assignment(np.asarray(inputs["edge_index"]))
    G = asg["G"]

    x_bf16_ext = np.zeros((N + 1, L), ml_dtypes.bfloat16)
    x_bf16_ext[:N] = x.astype(ml_dtypes.bfloat16)

    Wqs = np.concatenate([np.asarray(inputs["Wq"]), np.asarray(inputs["Wskip"])],
                         axis=1).astype(ml_dtypes.bfloat16)
    Wkv = np.concatenate([np.asarray(inputs["Wk"]), np.asarray(inputs["Wv"])],
                         axis=1).astype(ml_dtypes.bfloat16)
    GP = np.concatenate([np.asarray(inputs["pool_w"]),
                         np.asarray(inputs["gate_w"])], axis=1).astype(np.float32)

    in_maps = []
    for c in range(N_CORES):
        dst_rel_c = np.ascontiguousarray(asg["dst_rel"][c])
        # [128, NT*G] -> per-subtile row of 128 dst slots, broadcast to all
        # 128 partitions: dstt[p, (t, e)] = dst_rel_c[e, t]
        dstt_c = np.ascontiguousarray(
            np.broadcast_to(dst_rel_c.T.reshape(1, -1), (P, NT * G * P))
        ).astype(ml_dtypes.bfloat16)
        in_maps.append(stage_core_inputs(
            asg, x_bf16_ext, Wqs, Wkv, GP,
            dst_rel_c, dstt_c, asg["src_ids"][c], G))
    return asg, in_maps


def postprocess(results, asg, inputs):
    label = int(np.asarray(inputs["label"]))
    pool_b = np.asarray(inputs["pool_b"], np.float32)
    slot_node = asg["slot_node"]
    n_dummy = asg["n_dummy"]

    z = np.zeros(NCL, np.float64)
    yu = np.zeros((NCL, D), np.float64)
    for c in range(N_CORES):
        r = results[c]
        z += r["yu"][:, D].astype(np.float64) - float(n_dummy[c])
        yu += r["yu"][:, 0:D].astype(np.float64)

    attn = np.zeros((N, NCL), np.float32)
    h_full = np.zeros((N, D), np.float32)
    for c in range(N_CORES):
        r = results[c]
        mask = slot_node[c] >= 0
        nodes = slot_node[c][mask]
        attn[nodes] = (r["es"][mask].astype(np.float64) / z).astype(np.float32)
        h_full[nodes] = r["h"][mask]

    y = (yu / z[:, None]).astype(np.float32) + pool_b[None, :]
    A = attn[:, label]
    return y, attn, h_full, A


_CACHE = {}


def kernel(**inputs):
    asg, in_maps = prepare(inputs)
    G = asg["G"]
    if G not in _CACHE:
        _CACHE[G] = build_nc(G)
    nc = _CACHE[G]
    res = run_bass_kernel_spmd(nc, in_maps, core_ids=list(range(N_CORES)))
    return postprocess(res.results, asg, inputs)


if __name__ == "__main__":
    import reference
    inputs = {k: np.asarray(v) for k, v in reference.setup_inputs().items()}
    out = kernel(**inputs)
    print("y:", out[0].shape, "attn:", out[1].shape, "h:", out[2].shape,
          "A:", out[3].shape)
